# revision 1
# baseline (speedup 1.0000x reference)
"""Trainium2 Bass kernel for 4-head spatial self-attention (nn_Attention).

Reference computation (shapes hardcoded):
  x [4, 256, 64, 64] --1x1conv--> qkv [4, 384, 64, 64]
  per (batch, head): sim = (q*scale)^T k over c_head=32, softmax over j,
  out = attn @ v^T, then 1x1 out-projection back to 256 channels.

Sharding: 8 cores = 4 batches x 2 spatial halves (i-halves of 2048 tokens).
Each core computes k,v for its full batch and q for its i-half, producing a
complete [256, 2048] output slice; the host just concatenates. All cores run
an identical program (only the input data differs).

Per-core kernel strategy (v3):
  - sim is computed TRANSPOSED (j on partitions) via k^T q so the PV matmul
    needs no transpose; softmax max-subtraction is skipped (logits ~N(0,1),
    exp is safe, softmax is shift-invariant).
  - The PE is the critical resource and it executes its queue IN ORDER, so
    the emission is organized in conflict-free batches: per j-step, the 4 QK
    matmuls go to distinct 32-row PE tiles (they run concurrently), and the
    4 PV matmuls of j-step jt-2 follow (2x2 via distinct 32-col PE tiles).
    Lagging PV by 2 steps means its exp inputs are always long since ready,
    so the in-order PE never stalls at a PV waiting for an exp engine.
  - exp of the [128, 2, 512] sim tiles is SPLIT across two engines running
    in parallel: ~58% of head-pair tiles run true exp on the scalar engine
    (ACT, bf16 out); the rest run a Schraudolph-style exp on the vector
    engine (DVE): the QK psum already holds A*logit (A = 128*log2(e) folded
    into wq host-side), one tensor_scalar_add writes round(A*x + B) as
    int16, and those bits reinterpreted as bf16 ARE 2^(A*x/128 + cal) ~
    exp(x) (max rel err ~3.5%, mostly cancelled by softmax normalization;
    ACT undoes the A scaling for free via its scale= parameter).
  - The softmax denominator is fused into PV as a ones-column of vT (M=33
    matmuls, two heads col-packed per PSUM bank): bank rows [0:32]=even
    head, row 32 = its denominator, [64:96]=odd head, row 96 = its
    denominator. The head interleave is undone for free by zero-padded split
    output-projection weights built on the host; the output bias rides in a
    ones-row (row 32) of the pair-0 staging tile against a bias row added to
    woa host-side.
  - q/k/v projections run in bf16 (weights converted host-side; one bf16
    conversion copy of each x tile). QK runs in f32r (1 cycle/col; the q/k
    psum->sbuf copies are the required "rounded" producers).
  - Epilogue: reciprocal + attn*(1/d) on DVE, denominator-broadcast matmul +
    out-projection on PE, broadcast psum->sbuf copy on ACT, output DMA'd
    straight from PSUM. The epilogue of block ib is emitted after the first
    QK batch of block ib+1 so the exp engines never wait on it; vT copies
    run on ACT to balance engine load.
"""

import numpy as np

HEADS = 4
C_HEAD = 32
C_IN = 256
C_HID = 128
B = 4
NJ = 4096  # full token count (64*64)
NI = 2048  # per-core i-half
IB = 512  # i-block (PSUM bank width in fp32)
NJT = NJ // 128  # 32 j-tiles
NIB = NI // IB  # 4 i-blocks
P = 128
PV_LAG = 6
# j-steps where ACT takes both head-pairs (engine load balance: DVE carries
# the epilogue + conversion copies, ACT the vT copies)
ACT_BOTH_JTS = {2, 8, 14, 20, 26}

# Schraudolph exp constants: bits16 = round(A*x + SCHR_B) viewed as bf16
# approximates e^x. A is folded into wq host-side; ACT undoes it via scale.
LOG2E = 1.4426950408889634
SCHR_A = np.float32(128 * LOG2E)
SCHR_B = 16249.95

_STATE = {}


def _build_program(reps=1):
    import concourse.bacc as bacc
    import concourse.tile as tile
    from concourse import mybir

    F32 = mybir.dt.float32
    BF16 = mybir.dt.bfloat16

    nc = bacc.Bacc(None, target_bir_lowering=False)

    xkv = nc.declare_dram_parameter("xkv", [C_IN, NJ], F32, isOutput=False)
    xq = nc.declare_dram_parameter("xq", [C_IN, NI], F32, isOutput=False)
    wq = nc.declare_dram_parameter("wq_t", [C_IN, C_HID], BF16, isOutput=False)
    wk = nc.declare_dram_parameter("wk_t", [C_IN, C_HID], BF16, isOutput=False)
    wv = nc.declare_dram_parameter("wv_t", [C_IN, C_HID], BF16, isOutput=False)
    woa = nc.declare_dram_parameter("woa_t", [C_HID, C_IN], F32, isOutput=False)
    wob = nc.declare_dram_parameter("wob_t", [C_HID, C_IN], F32, isOutput=False)
    out = nc.declare_dram_parameter("out", [C_IN, NI], F32, isOutput=True)

    with tile.TileContext(nc) as tc:
        with (
            tc.tile_pool(name="consts", bufs=1) as consts,
            tc.tile_pool(name="xpool", bufs=1) as xpool,
            tc.tile_pool(name="qkv", bufs=1) as qkv,
            tc.tile_pool(name="epool", bufs=6) as epool,
            tc.tile_pool(name="misc", bufs=2) as misc,
            tc.tile_pool(name="aofix", bufs=1) as aofix,
            tc.tile_pool(name="psim", bufs=3, space="PSUM") as psim,
            tc.tile_pool(name="ppv", bufs=1, space="PSUM") as ppv,
        ):
            # --- constants / weights (loaded once) ---
            wq_t = consts.tile([P, 2, C_HID], BF16)
            nc.sync.dma_start(out=wq_t, in_=wq[:].rearrange("(t p) m -> p t m", p=P))
            wk_t = consts.tile([P, 2, C_HID], BF16)
            nc.sync.dma_start(out=wk_t, in_=wk[:].rearrange("(t p) m -> p t m", p=P))
            wv_t = consts.tile([P, 2, C_HID], BF16)
            nc.sync.dma_start(out=wv_t, in_=wv[:].rearrange("(t p) m -> p t m", p=P))
            F32R = mybir.dt.float32r
            woa_f = consts.tile([P, C_IN], F32)
            nc.sync.dma_start(out=woa_f, in_=woa[:])
            wob_f = consts.tile([P, C_IN], F32)
            nc.sync.dma_start(out=wob_f, in_=wob[:])
            # f32r copies: rounded producers so the out-projection matmuls
            # run at 1 cycle/col with a cheap weight load (fp32 stationaries
            # self-load inside the matmul at 4 cycles/col)
            woa_t = consts.tile([P, C_IN], F32R)
            nc.vector.tensor_copy(woa_t, woa_f)
            wob_t = consts.tile([P, C_IN], F32R)
            nc.vector.tensor_copy(wob_t, wob_f)
            # memset cannot write f32r; bounce constants through an f32
            # scratch and tensor_copy (a legal f32r rounding producer)
            zo = consts.tile([P, IB], F32)
            nc.vector.memset(zo, 0.0)
            ones_row = consts.tile([P, C_HEAD], F32)
            nc.vector.memset(ones_row, 1.0)

            # attention-out staging tiles: the zero rows (and the bias ones-
            # row at pair-0 row 32) are set once and survive every i-block /
            # rep (the loop only writes the data rows)
            nc.vector.memset(zo, 0.0)
            ao = []
            for pair in range(2):
                ao_t = aofix.tile([P, IB], F32R, tag=f"ao{pair}")
                for hh in range(2):
                    zs = slice(64 * hh + 32, 64 * hh + 64)
                    nc.vector.tensor_copy(ao_t[zs, :], zo[zs, :])
                ao.append(ao_t)
            nc.vector.memset(zo[32:33, :], 1.0)
            nc.vector.tensor_copy(ao[0][32:33, :], zo[32:33, :])  # bias row

            env = dict(
                xkv=xkv, xq=xq, out=out,
                wq_t=wq_t, wk_t=wk_t, wv_t=wv_t, woa_t=woa_t, wob_t=wob_t,
                ones_row=ones_row, ao=ao,
                xpool=xpool, qkv=qkv, epool=epool, misc=misc,
                psim=psim, ppv=ppv,
            )
            if reps == 1:
                _emit_body(nc, tc, mybir, env)
            else:
                with tc.For_i(0, reps, 1, staggered_reset=True):
                    _emit_body(nc, tc, mybir, env)

    nc.compile()
    return nc


def _emit_body(nc, tc, mybir, env):
    F32 = mybir.dt.float32
    BF16 = mybir.dt.bfloat16
    F32R = mybir.dt.float32r
    I16 = mybir.dt.int16
    EXP = mybir.ActivationFunctionType.Exp

    xkv, xq, out = env["xkv"], env["xq"], env["out"]
    wq_t, wk_t, wv_t = env["wq_t"], env["wk_t"], env["wv_t"]
    woa_t, wob_t = env["woa_t"], env["wob_t"]
    ones_row, ao = env["ones_row"], env["ao"]
    xpool, qkv, epool, misc = env["xpool"], env["qkv"], env["epool"], env["misc"]
    psim, ppv = env["psim"], env["ppv"]

    xq_t = xpool.tile([P, 2, NI], F32)
    nc.sync.dma_start(out=xq_t, in_=xq[:].rearrange("(t p) n -> p t n", p=P))
    xkv_t = xpool.tile([P, 2, NJ], F32)
    nc.sync.dma_start(out=xkv_t, in_=xkv[:].rearrange("(t p) n -> p t n", p=P))

    xq_bf = qkv.tile([P, 2, NI], BF16)
    nc.vector.tensor_copy(xq_bf, xq_t)
    xkv_bf = qkv.tile([P, 2, NJ], BF16)
    nc.vector.tensor_copy(xkv_bf, xkv_t)
    # q/k in bf16: a bf16 stationary gets a separate (FWL-fast) LDWEIGHTS,
    # unlike f32r whose weight load runs inside the matmul at 4 cycles/col
    q_t = qkv.tile([P, NI], BF16)
    k_t = qkv.tile([P, NJ], BF16)
    # vT layout: [j-part, j-tile, head, 34]; cols 0:32 = v^T, col 32 = ones
    # (fused softmax denominator), col 33 = padding.
    vT_t = qkv.tile([P, NJT, HEADS, 34], BF16)
    nc.gpsimd.memset(vT_t[:, :, :, 32:34], 1.0)

    # --- projections ---
    # q[c_hid, i] = wq_t.T @ xq ; softmax scale * SCHR_A folded host-side
    for c0 in range(0, NI, IB):
        pq = psim.tile([P, IB], F32, tag="sim")
        for t in range(2):
            nc.tensor.matmul(
                pq, wq_t[:, t, :], xq_bf[:, t, c0 : c0 + IB],
                start=(t == 0), stop=(t == 1),
            )
        nc.vector.tensor_copy(q_t[:, c0 : c0 + IB], pq)

    def emit_k_chunk(c0):
        pk = psim.tile([P, IB], F32, tag="sim")
        for t in range(2):
            nc.tensor.matmul(
                pk, wk_t[:, t, :], xkv_bf[:, t, c0 : c0 + IB],
                start=(t == 0), stop=(t == 1),
            )
        nc.vector.tensor_copy(k_t[:, c0 : c0 + IB], pk)

    # vT[j, c_hid] = x_tile.T @ wv_t (x stationary, bf16); copy on ACT
    def emit_vt(jt):
        pv_ = psim.tile([P, C_HID], F32, tag="sim")
        for t in range(2):
            nc.tensor.matmul(
                pv_, xkv_bf[:, t, jt * P : (jt + 1) * P], wv_t[:, t, :],
                start=(t == 0), stop=(t == 1),
            )
        nc.scalar.copy(
            vT_t[:, jt, :, 0:32],
            pv_[:].rearrange("p (h c) -> p h c", h=HEADS),
        )

    # --- attention ---
    def emit_pv(ib, jt, e_pair, pv_ps):
        for pair in range(2):
            e_t = e_pair[pair]
            for hh in range(2):
                h = pair * 2 + hh
                nc.tensor.matmul(
                    pv_ps[pair][64 * hh : 64 * hh + 33, :],
                    vT_t[:, jt, h, 0:33], e_t[:, hh, :],
                    start=(jt == 0), stop=(jt == NJT - 1),
                    tile_position=(0, 64 * hh),
                )

    def emit_epilogue(ib, pvsb):
        isl = slice(ib * IB, (ib + 1) * IB)
        # normalization: head rows scale by 1/denominator per column
        bc_ps = psim.tile([P, 2, IB], F32, tag="sim")
        bc_sb = misc.tile([P, 2, IB], F32, tag="bcsb")
        for pair in range(2):
            recip = misc.tile([P, IB], F32, tag=f"recip{pair}")
            nc.vector.reciprocal(recip[0:97, :], pvsb[pair][0:97, :])
            for hh in range(2):
                nc.tensor.matmul(
                    bc_ps[64 * hh : 64 * hh + 32, pair, :],
                    ones_row[32 + 64 * hh : 33 + 64 * hh, :],
                    recip[32 + 64 * hh : 33 + 64 * hh, :],
                    start=True, stop=True,
                    tile_position=(32 + 64 * hh, 64 * hh),
                )
            nc.scalar.copy(bc_sb[:, pair, :], bc_ps[:, pair, :])
            for hh in range(2):
                rs = slice(64 * hh, 64 * hh + 32)
                nc.vector.tensor_mul(
                    ao[pair][rs, :], pvsb[pair][rs, :], bc_sb[rs, pair, :]
                )
        # output projection (zero-padded split weights undo the head
        # interleave; bias rides on ao[0] row 32 x woa bias row)
        pr_ps = psim.tile([P, 2, IB], F32, tag="sim")
        o_t = misc.tile([P, 2, IB], F32, tag="o")
        for ot in range(2):
            osl = slice(ot * P, (ot + 1) * P)
            nc.tensor.matmul(
                pr_ps[:, ot, :], woa_t[:, osl], ao[0], start=True, stop=False
            )
            nc.tensor.matmul(
                pr_ps[:, ot, :], wob_t[:, osl], ao[1], start=False, stop=True
            )
            if ot == 0:
                nc.scalar.copy(o_t[:, ot, :], pr_ps[:, ot, :])
            else:
                nc.vector.tensor_copy(o_t[:, ot, :], pr_ps[:, ot, :])
            nc.sync.dma_start(
                out=out[:].rearrange("(t p) n -> p t n", p=P)[:, ot, isl],
                in_=o_t[:, ot, :],
            )

    def pop_pv():
        # lagged PV batch (dependencies long satisfied -> no PE stall); on
        # the last j-tile of a block, immediately evict the finished PV
        # accumulators to SBUF via DMA (its queue is independent of the busy
        # compute engines) so the banks are free for the next block.
        pib, pjt, pe, pps = pv_queue.pop(0)
        emit_pv(pib, pjt, pe, pps)
        if pjt == NJT - 1:
            pvsb = []
            for pair in range(2):
                sb = misc.tile([P, IB], F32, tag=f"pvsb{pair}")
                if pair == 0:
                    nc.scalar.copy(sb, pps[pair][:])
                else:
                    nc.vector.tensor_copy(sb, pps[pair][:])
                pvsb.append(sb)
            return (pib, pvsb)
        return None

    pending_epilogue = None
    pv_queue = []
    for ib in range(NIB):
        isl = slice(ib * IB, (ib + 1) * IB)
        pv_a = ppv.tile([P, IB], F32, tag="pv0")
        pv_b = ppv.tile([P, IB], F32, tag="pv1")
        pv_ps = [pv_a, pv_b]
        for jt in range(NJT):
            jsl = slice(jt * P, (jt + 1) * P)
            if ib == 0:
                # stream the k / vT projections just ahead of first use
                if jt % 4 == 0:
                    emit_k_chunk(jt * P)
                emit_vt(jt)
            # QK batch: 4 concurrent row-tiles
            sims = []
            for pair in range(2):
                sim = psim.tile([P, 2, IB], F32, tag="sim")
                for hh in range(2):
                    h = pair * 2 + hh
                    hsl = slice(h * C_HEAD, (h + 1) * C_HEAD)
                    nc.tensor.matmul(
                        sim[:, hh, :], k_t[hsl, jsl], q_t[hsl, isl],
                        start=True, stop=True,
                        tile_position=(h * C_HEAD, 0),
                    )
                sims.append(sim)
            # exp batch, split across ACT / DVE
            e_pair = []
            for pair in range(2):
                if pair == 0 or jt in ACT_BOTH_JTS:
                    e_t = epool.tile([P, 2, IB], BF16, tag="ea")
                    nc.scalar.activation(
                        e_t, sims[pair], EXP, scale=float(1.0 / SCHR_A)
                    )
                else:
                    e_i = epool.tile([P, 2, IB], I16, tag="ed")
                    nc.vector.tensor_scalar_add(e_i, sims[pair], float(SCHR_B))
                    e_t = e_i[:].bitcast(BF16)
                e_pair.append(e_t)
            pv_queue.append((ib, jt, e_pair, pv_ps))
            if len(pv_queue) > PV_LAG:
                done = pop_pv()
                if done is not None:
                    pending_epilogue = done
            # previous block's epilogue PE ops go in late enough that their
            # cross-engine waits (reciprocal on DVE behind queued exps) can
            # never stall the PE queue head
            if jt == 8 and pending_epilogue is not None:
                emit_epilogue(*pending_epilogue)
                pending_epilogue = None
    while pv_queue:
        done = pop_pv()
        if done is not None:
            pending_epilogue = done
    emit_epilogue(*pending_epilogue)


def _get_nc(reps=1):
    key = ("nc", reps)
    if key not in _STATE:
        _STATE[key] = _build_program(reps)
    return _STATE[key]


def _make_in_maps(x, w_qkv, w_out, b_out):
    import ml_dtypes

    x = np.ascontiguousarray(x, dtype=np.float32)
    w_qkv = np.asarray(w_qkv, dtype=np.float32)
    w_out = np.asarray(w_out, dtype=np.float32)
    b_out = np.asarray(b_out, dtype=np.float32)
    scale = np.float32(C_HEAD**-0.5) * SCHR_A
    wo_t = w_out.T  # [c_hid, c_in]
    woa = np.zeros((C_HID, C_IN), np.float32)
    wob = np.zeros((C_HID, C_IN), np.float32)
    woa[0:32] = wo_t[0:32]  # head 0
    woa[64:96] = wo_t[32:64]  # head 1
    woa[32] = b_out  # bias row (matches ao[0] ones-row 32)
    wob[0:32] = wo_t[64:96]  # head 2
    wob[64:96] = wo_t[96:128]  # head 3

    def bf(a):
        return np.ascontiguousarray(a).astype(ml_dtypes.bfloat16)

    shared = {
        "wq_t": bf((w_qkv[0:C_HID] * scale).T),
        "wk_t": bf(w_qkv[C_HID : 2 * C_HID].T),
        "wv_t": bf(w_qkv[2 * C_HID : 3 * C_HID].T),
        "woa_t": woa,
        "wob_t": wob,
    }
    in_maps = []
    for c in range(8):
        b, half = divmod(c, 2)
        xkv = np.ascontiguousarray(x[b].reshape(C_IN, NJ))
        xq = np.ascontiguousarray(xkv[:, half * NI : (half + 1) * NI])
        in_maps.append({"xkv": xkv, "xq": xq, **shared})
    return in_maps


def _assemble(results):
    out = np.empty((B, C_IN, NJ), np.float32)
    for c in range(8):
        b, half = divmod(c, 2)
        out[b][:, half * NI : (half + 1) * NI] = results[c]["out"]
    return out.reshape(B, C_IN, 64, 64)


def _run(in_maps, reps=1, **kwargs):
    from concourse.bass_utils import run_bass_kernel_spmd

    return run_bass_kernel_spmd(
        _get_nc(reps), in_maps, core_ids=list(range(8)), **kwargs
    )


def kernel(x, w_qkv, w_out, b_out):
    res = _run(_make_in_maps(x, w_qkv, w_out, b_out))
    return _assemble(res.results)



# revision 4
# speedup vs baseline: 1.3489x; 1.3489x over previous
"""Trainium2 Bass kernel for 4-head spatial self-attention (nn_Attention).

Reference computation (shapes hardcoded):
  x [4, 256, 64, 64] --1x1conv--> qkv [4, 384, 64, 64]
  per (batch, head): sim = (q*scale)^T k over c_head=32, softmax over j,
  out = attn @ v^T, then 1x1 out-projection back to 256 channels.

Sharding: 8 cores = 4 batches x 2 spatial halves (i-halves of 2048 tokens).
Each core computes k,v for its full batch and q for its i-half, producing a
complete [256, 2048] output slice; the host just concatenates. All cores run
an identical program (only the input data differs).

Per-core kernel strategy (v4):
  - sim is computed TRANSPOSED (j on partitions) via k^T q so the PV matmul
    needs no transpose; softmax max-subtraction is skipped (logits ~N(0,1),
    exp is safe, softmax is shift-invariant).
  - The exp of the [j, i] sim tiles is the dominant cost (every sim element
    must cross ACT or DVE once: PE cannot read PSUM, DMA has no PSUM route).
    The work is split between ACT (true exp, bf16 out) and DVE (Schraudolph
    exp: one tensor_scalar_add writes round(A*x + B) as int16; those bits
    reinterpreted as bf16 ARE ~exp(x); A folded into wq host-side, undone on
    ACT via its scale= parameter). The split is parameterized (N_ACT_BOTH
    per block) and tuned by measurement.
  - The PE executes its queue IN ORDER; emission is organized so nothing at
    the PE queue head ever waits on a slow engine: per j-step the 4 QK
    matmuls go to distinct 32-row PE tiles (concurrent), and the 4 PV
    matmuls of j-step jt-PV_LAG follow (2x2 via distinct 32-col PE tiles),
    so PV's exp inputs are long since ready.
  - The softmax denominator is fused into PV as a ones-column of vT (M=33
    matmuls, two heads col-packed per PSUM bank): bank rows [0:32]=even
    head, row 32 = its denominator, [64:96]=odd head, row 96 = its
    denominator. The head interleave is undone for free by zero-padded split
    output-projection weights built on the host; the output bias rides in a
    ones-row (row 32) of the pair-0 staging tile against a bias row added to
    woa host-side.
  - x arrives from the host ALREADY in bf16 (no on-chip conversion); its DMA
    is chunked so the k/v projections can start before the full load lands.
  - v^T eviction is batched 4 j-tiles per ACT copy (FD=512, amortizes the
    fixed PSUM-access cost); k eviction alternates ACT/DVE.
  - The epilogue is STAGGERED across j-steps 7..11 of the next block so the
    strict-FIFO ACT/DVE queues never head-block on a cross-engine wait:
    reciprocal_approx_fast (~5x faster than exact reciprocal) at jt=7, PE
    denominator-broadcast at jt=8, its ACT eviction at jt=9, the attn*(1/d)
    muls at jt=10, output projection + eviction + DMA at jt=11.
"""

import numpy as np

HEADS = 4
C_HEAD = 32
C_IN = 256
C_HID = 128
B = 4
NJ = 4096  # full token count (64*64)
NI = 2048  # per-core i-half
IB = 512  # i-block (PSUM bank width in fp32)
NJT = NJ // 128  # 32 j-tiles
NIB = NI // IB  # 4 i-blocks
P = 128
PV_LAG = 6

# exp split: number of j-steps per block where ACT takes BOTH head-pairs
# (otherwise pair0->ACT, pair1->DVE). Tuned by measurement.
N_ACT_BOTH = 5
N_ACT_BOTH_B0 = 7  # block 0 carries extra DVE work (q/k evictions)

# Schraudolph exp constants: bits16 = round(A*x + SCHR_B) viewed as bf16
# approximates e^x. A is folded into wq host-side; ACT undoes it via scale.
LOG2E = 1.4426950408889634
SCHR_A = np.float32(128 * LOG2E)
SCHR_B = 16249.95

_STATE = {}


def _act_both_jts(ib):
    n = N_ACT_BOTH_B0 if ib == 0 else N_ACT_BOTH
    if n <= 0:
        return set()
    # spread evenly, avoiding the first two j-steps (pipeline warm-up)
    return {2 + int(k * 30 / n) for k in range(n)}


def _build_program(reps=1):
    import concourse.bacc as bacc
    import concourse.tile as tile
    from concourse import mybir

    F32 = mybir.dt.float32
    BF16 = mybir.dt.bfloat16

    nc = bacc.Bacc(None, target_bir_lowering=False)

    xkv = nc.declare_dram_parameter("xkv", [C_IN, NJ], BF16, isOutput=False)
    xq = nc.declare_dram_parameter("xq", [C_IN, NI], BF16, isOutput=False)
    wq = nc.declare_dram_parameter("wq_t", [C_IN, C_HID], BF16, isOutput=False)
    wk = nc.declare_dram_parameter("wk_t", [C_IN, C_HID], BF16, isOutput=False)
    wv = nc.declare_dram_parameter("wv_t", [C_IN, C_HID], BF16, isOutput=False)
    woa = nc.declare_dram_parameter("woa_t", [C_HID, C_IN], F32, isOutput=False)
    wob = nc.declare_dram_parameter("wob_t", [C_HID, C_IN], F32, isOutput=False)
    out = nc.declare_dram_parameter("out", [C_IN, NI], F32, isOutput=True)

    with tile.TileContext(nc) as tc:
        with (
            tc.tile_pool(name="consts", bufs=1) as consts,
            tc.tile_pool(name="xpool", bufs=1) as xpool,
            tc.tile_pool(name="qkv", bufs=1) as qkv,
            tc.tile_pool(name="epool", bufs=12) as epool,
            tc.tile_pool(name="misc", bufs=2) as misc,
            tc.tile_pool(name="aofix", bufs=1) as aofix,
            tc.tile_pool(name="psim", bufs=3, space="PSUM") as psim,
            tc.tile_pool(name="ppv", bufs=1, space="PSUM") as ppv,
        ):
            # --- constants / weights (loaded once) ---
            wq_t = consts.tile([P, 2, C_HID], BF16)
            nc.sync.dma_start(out=wq_t, in_=wq[:].rearrange("(t p) m -> p t m", p=P))
            wk_t = consts.tile([P, 2, C_HID], BF16)
            nc.sync.dma_start(out=wk_t, in_=wk[:].rearrange("(t p) m -> p t m", p=P))
            wv_t = consts.tile([P, 2, C_HID], BF16)
            nc.sync.dma_start(out=wv_t, in_=wv[:].rearrange("(t p) m -> p t m", p=P))
            F32R = mybir.dt.float32r
            woa_f = consts.tile([P, C_IN], F32)
            nc.sync.dma_start(out=woa_f, in_=woa[:])
            wob_f = consts.tile([P, C_IN], F32)
            nc.sync.dma_start(out=wob_f, in_=wob[:])
            # f32r copies: rounded producers so the out-projection matmuls
            # run at 1 cycle/col with a cheap weight load (fp32 stationaries
            # self-load inside the matmul at 4 cycles/col)
            woa_t = consts.tile([P, C_IN], F32R)
            nc.vector.tensor_copy(woa_t, woa_f)
            wob_t = consts.tile([P, C_IN], F32R)
            nc.vector.tensor_copy(wob_t, wob_f)
            # memset cannot write f32r; bounce constants through an f32
            # scratch and tensor_copy (a legal f32r rounding producer)
            zo = consts.tile([P, IB], F32)
            nc.vector.memset(zo, 0.0)
            ones_row = consts.tile([P, C_HEAD], F32)
            nc.vector.memset(ones_row, 1.0)

            # attention-out staging tiles: the zero rows (and the bias ones-
            # row at pair-0 row 32) are set once and survive every i-block /
            # rep (the loop only writes the data rows)
            ao = []
            for pair in range(2):
                ao_t = aofix.tile([P, IB], F32R, tag=f"ao{pair}")
                for hh in range(2):
                    zs = slice(64 * hh + 32, 64 * hh + 64)
                    nc.vector.tensor_copy(ao_t[zs, :], zo[zs, :])
                ao.append(ao_t)
            nc.vector.memset(zo[32:33, :], 1.0)
            nc.vector.tensor_copy(ao[0][32:33, :], zo[32:33, :])  # bias row

            env = dict(
                xkv=xkv, xq=xq, out=out,
                wq_t=wq_t, wk_t=wk_t, wv_t=wv_t, woa_t=woa_t, wob_t=wob_t,
                ones_row=ones_row, ao=ao,
                xpool=xpool, qkv=qkv, epool=epool, misc=misc,
                psim=psim, ppv=ppv,
            )
            if reps == 1:
                _emit_body(nc, tc, mybir, env)
            else:
                with tc.For_i(0, reps, 1, staggered_reset=True):
                    _emit_body(nc, tc, mybir, env)

    nc.compile()
    return nc


def _emit_body(nc, tc, mybir, env):
    F32 = mybir.dt.float32
    BF16 = mybir.dt.bfloat16
    F32R = mybir.dt.float32r
    I16 = mybir.dt.int16
    EXP = mybir.ActivationFunctionType.Exp

    xkv, xq, out = env["xkv"], env["xq"], env["out"]
    wq_t, wk_t, wv_t = env["wq_t"], env["wk_t"], env["wv_t"]
    woa_t, wob_t = env["woa_t"], env["wob_t"]
    ones_row, ao = env["ones_row"], env["ao"]
    xpool, qkv, epool, misc = env["xpool"], env["qkv"], env["epool"], env["misc"]
    psim, ppv = env["psim"], env["ppv"]

    # x arrives in bf16; chunked DMAs so projections start before the full
    # load lands
    xq_t = xpool.tile([P, 2, NI], BF16)
    for c0 in range(0, NI, 1024):
        nc.sync.dma_start(
            out=xq_t[:, :, c0 : c0 + 1024],
            in_=xq[:, c0 : c0 + 1024].rearrange("(t p) n -> p t n", p=P),
        )
    xkv_t = xpool.tile([P, 2, NJ], BF16)
    for c0 in range(0, NJ, 1024):
        nc.sync.dma_start(
            out=xkv_t[:, :, c0 : c0 + 1024],
            in_=xkv[:, c0 : c0 + 1024].rearrange("(t p) n -> p t n", p=P),
        )

    # q/k in bf16: a bf16 stationary gets a separate (FWL-fast) LDWEIGHTS,
    # unlike f32r whose weight load runs inside the matmul at 4 cycles/col
    q_t = qkv.tile([P, NI], BF16)
    k_t = qkv.tile([P, NJ], BF16)
    # vT layout: [j-part, j-tile, head, 34]; cols 0:32 = v^T, col 32 = ones
    # (fused softmax denominator), col 33 = padding.
    vT_t = qkv.tile([P, NJT, HEADS, 34], BF16)
    nc.gpsimd.memset(vT_t[:, :, :, 32:34], 1.0)

    # --- projections ---
    # q[c_hid, i] = wq_t.T @ xq ; softmax scale * SCHR_A folded host-side
    for c0 in range(0, NI, IB):
        pq = psim.tile([P, IB], F32, tag="sim")
        for t in range(2):
            nc.tensor.matmul(
                pq, wq_t[:, t, :], xq_t[:, t, c0 : c0 + IB],
                start=(t == 0), stop=(t == 1),
            )
        nc.vector.tensor_copy(q_t[:, c0 : c0 + IB], pq)

    def emit_k_chunk(c0):
        pk = psim.tile([P, IB], F32, tag="sim")
        for t in range(2):
            nc.tensor.matmul(
                pk, wk_t[:, t, :], xkv_t[:, t, c0 : c0 + IB],
                start=(t == 0), stop=(t == 1),
            )
        if (c0 // IB) % 2 == 0:
            nc.scalar.copy(k_t[:, c0 : c0 + IB], pk)
        else:
            nc.vector.tensor_copy(k_t[:, c0 : c0 + IB], pk)

    # vT[j, c_hid] = x_tile.T @ wv_t (x stationary, bf16); 4 j-tiles batched
    # per psum bank so one ACT copy (FD=512) evicts them all
    def emit_vt4(g):
        pv_ = psim.tile([P, 4, C_HID], F32, tag="sim")
        for j in range(4):
            jt = g * 4 + j
            for t in range(2):
                nc.tensor.matmul(
                    pv_[:, j, :], xkv_t[:, t, jt * P : (jt + 1) * P], wv_t[:, t, :],
                    start=(t == 0), stop=(t == 1),
                )
        nc.scalar.copy(
            vT_t[:, g * 4 : (g + 1) * 4, :, 0:32],
            pv_[:].rearrange("p j (h c) -> p j h c", h=HEADS),
        )

    # --- attention ---
    def emit_pv(ib, jt, e_pair, pv_ps):
        for pair in range(2):
            e_t = e_pair[pair]
            for hh in range(2):
                h = pair * 2 + hh
                nc.tensor.matmul(
                    pv_ps[pair][64 * hh : 64 * hh + 33, :],
                    vT_t[:, jt, h, 0:33], e_t[:, hh, :],
                    start=(jt == 0), stop=(jt == NJT - 1),
                    tile_position=(0, 64 * hh),
                )

    # --- epilogue, staggered into pieces so ACT/DVE FIFOs never head-block
    def epi_recip(st):
        for pair in range(2):
            recip = misc.tile([P, IB], F32, tag=f"recip{pair}")
            nc.vector.reciprocal_approx_fast(recip[0:97, :], st["pvsb"][pair][0:97, :])
            st[f"recip{pair}"] = recip

    def epi_bcast(st):
        # normalization: head rows scale by 1/denominator per column
        bc_ps = psim.tile([P, 2, IB], F32, tag="sim")
        for pair in range(2):
            for hh in range(2):
                nc.tensor.matmul(
                    bc_ps[64 * hh : 64 * hh + 32, pair, :],
                    ones_row[32 + 64 * hh : 33 + 64 * hh, :],
                    st[f"recip{pair}"][32 + 64 * hh : 33 + 64 * hh, :],
                    start=True, stop=True,
                    tile_position=(32 + 64 * hh, 64 * hh),
                )
        st["bc_ps"] = bc_ps

    def epi_bccopy(st):
        bc_sb = misc.tile([P, 2, IB], F32, tag="bcsb")
        nc.scalar.copy(bc_sb, st["bc_ps"])
        st["bc_sb"] = bc_sb

    def epi_mul(st):
        for pair in range(2):
            for hh in range(2):
                rs = slice(64 * hh, 64 * hh + 32)
                nc.vector.tensor_mul(
                    ao[pair][rs, :], st["pvsb"][pair][rs, :], st["bc_sb"][rs, pair, :]
                )

    def epi_out(st):
        # output projection (zero-padded split weights undo the head
        # interleave; bias rides on ao[0] row 32 x woa bias row)
        ib = st["ib"]
        isl = slice(ib * IB, (ib + 1) * IB)
        pr_ps = psim.tile([P, 2, IB], F32, tag="sim")
        o_t = misc.tile([P, 2, IB], F32, tag="o")
        for ot in range(2):
            osl = slice(ot * P, (ot + 1) * P)
            nc.tensor.matmul(
                pr_ps[:, ot, :], woa_t[:, osl], ao[0], start=True, stop=False
            )
            nc.tensor.matmul(
                pr_ps[:, ot, :], wob_t[:, osl], ao[1], start=False, stop=True
            )
            if ot == 0:
                nc.scalar.copy(o_t[:, ot, :], pr_ps[:, ot, :])
            else:
                nc.vector.tensor_copy(o_t[:, ot, :], pr_ps[:, ot, :])
            nc.sync.dma_start(
                out=out[:].rearrange("(t p) n -> p t n", p=P)[:, ot, isl],
                in_=o_t[:, ot, :],
            )

    # stage placement accounts for the consumer engine's FIFO backlog (~1
    # j-step) so no engine ever stalls at its queue head waiting on another:
    # recips right after the pvsb eviction lands; the PE broadcast 4 j-steps
    # later (DVE surely through the recips by then); out-projection 5
    # j-steps after the muls are queued.
    EPI_STAGES = [epi_recip, epi_bcast, epi_bccopy, epi_mul, epi_out]
    EPI_JTS = {6: 0, 10: 1, 11: 2, 12: 3, 17: 4}

    def pop_pv():
        # lagged PV batch (dependencies long satisfied -> no PE stall); on
        # the last j-tile of a block, immediately evict the finished PV
        # accumulators to SBUF so the banks are free for the next block.
        pib, pjt, pe, pps = pv_queue.pop(0)
        emit_pv(pib, pjt, pe, pps)
        if pjt == NJT - 1:
            pvsb = []
            for pair in range(2):
                sb = misc.tile([P, IB], F32, tag=f"pvsb{pair}")
                if pair == 0:
                    nc.scalar.copy(sb, pps[pair][:])
                else:
                    nc.vector.tensor_copy(sb, pps[pair][:])
                pvsb.append(sb)
            return {"ib": pib, "pvsb": pvsb}
        return None

    pending_epi = None
    pv_queue = []
    for ib in range(NIB):
        isl = slice(ib * IB, (ib + 1) * IB)
        act_both = _act_both_jts(ib)
        pv_a = ppv.tile([P, IB], F32, tag="pv0")
        pv_b = ppv.tile([P, IB], F32, tag="pv1")
        pv_ps = [pv_a, pv_b]
        for jt in range(NJT):
            jsl = slice(jt * P, (jt + 1) * P)
            if ib == 0:
                # stream the k / vT projections just ahead of first use
                if jt % 4 == 0:
                    emit_k_chunk(jt * P)
                    emit_vt4(jt // 4)
            # QK batch: 4 concurrent row-tiles
            sims = []
            for pair in range(2):
                sim = psim.tile([P, 2, IB], F32, tag="sim")
                for hh in range(2):
                    h = pair * 2 + hh
                    hsl = slice(h * C_HEAD, (h + 1) * C_HEAD)
                    nc.tensor.matmul(
                        sim[:, hh, :], k_t[hsl, jsl], q_t[hsl, isl],
                        start=True, stop=True,
                        tile_position=(h * C_HEAD, 0),
                    )
                sims.append(sim)
            # exp batch, split across ACT / DVE
            e_pair = []
            for pair in range(2):
                if pair == 0 or jt in act_both:
                    e_t = epool.tile([P, 2, IB], BF16, tag="ea")
                    nc.scalar.activation(
                        e_t, sims[pair], EXP, scale=float(1.0 / SCHR_A)
                    )
                else:
                    e_i = epool.tile([P, 2, IB], I16, tag="ed")
                    nc.vector.tensor_scalar_add(e_i, sims[pair], float(SCHR_B))
                    e_t = e_i[:].bitcast(BF16)
                e_pair.append(e_t)
            pv_queue.append((ib, jt, e_pair, pv_ps))
            if len(pv_queue) > PV_LAG:
                done = pop_pv()
                if done is not None:
                    pending_epi = done
            # previous block's epilogue pieces go in one-per-j-step, late
            # enough that every cross-engine wait is satisfied before the
            # instruction reaches its strict-FIFO queue head
            if jt in EPI_JTS and pending_epi is not None:
                EPI_STAGES[EPI_JTS[jt]](pending_epi)
                if EPI_JTS[jt] == len(EPI_STAGES) - 1:
                    pending_epi = None
    while pv_queue:
        done = pop_pv()
        if done is not None:
            pending_epi = done
    for stage in EPI_STAGES:
        stage(pending_epi)


def _get_nc(reps=1):
    key = ("nc", reps)
    if key not in _STATE:
        _STATE[key] = _build_program(reps)
    return _STATE[key]


def _make_in_maps(x, w_qkv, w_out, b_out):
    import ml_dtypes

    x = np.ascontiguousarray(x, dtype=np.float32)
    w_qkv = np.asarray(w_qkv, dtype=np.float32)
    w_out = np.asarray(w_out, dtype=np.float32)
    b_out = np.asarray(b_out, dtype=np.float32)
    scale = np.float32(C_HEAD**-0.5) * SCHR_A
    wo_t = w_out.T  # [c_hid, c_in]
    woa = np.zeros((C_HID, C_IN), np.float32)
    wob = np.zeros((C_HID, C_IN), np.float32)
    woa[0:32] = wo_t[0:32]  # head 0
    woa[64:96] = wo_t[32:64]  # head 1
    woa[32] = b_out  # bias row (matches ao[0] ones-row 32)
    wob[0:32] = wo_t[64:96]  # head 2
    wob[64:96] = wo_t[96:128]  # head 3

    def bf(a):
        return np.ascontiguousarray(a).astype(ml_dtypes.bfloat16)

    shared = {
        "wq_t": bf((w_qkv[0:C_HID] * scale).T),
        "wk_t": bf(w_qkv[C_HID : 2 * C_HID].T),
        "wv_t": bf(w_qkv[2 * C_HID : 3 * C_HID].T),
        "woa_t": woa,
        "wob_t": wob,
    }
    in_maps = []
    for c in range(8):
        b, half = divmod(c, 2)
        xkv = bf(x[b].reshape(C_IN, NJ))
        xq = np.ascontiguousarray(xkv[:, half * NI : (half + 1) * NI])
        in_maps.append({"xkv": xkv, "xq": xq, **shared})
    return in_maps


def _assemble(results):
    out = np.empty((B, C_IN, NJ), np.float32)
    for c in range(8):
        b, half = divmod(c, 2)
        out[b][:, half * NI : (half + 1) * NI] = results[c]["out"]
    return out.reshape(B, C_IN, 64, 64)


def _run(in_maps, reps=1, **kwargs):
    from concourse.bass_utils import run_bass_kernel_spmd

    return run_bass_kernel_spmd(
        _get_nc(reps), in_maps, core_ids=list(range(8)), **kwargs
    )


def kernel(x, w_qkv, w_out, b_out):
    res = _run(_make_in_maps(x, w_qkv, w_out, b_out))
    return _assemble(res.results)
